# revision 26
# baseline (speedup 1.0000x reference)
"""Differentiable SSN (superpixel) kernel for Trainium2, 8 NeuronCores.

Computes 3 iterations of {cdist -> softmax -> soft center update} for
B=4, N=65536 pixels, M=256 centers, D=18 (16 features + 2 coords).

Strategy:
  - pixels sharded 8 ways across cores (each core: 8192 pixels x 4 batches)
  - distance^2 via one fp32r matmul per 128-pixel tile using a 21-dim
    augmentation [p, pnorm_hi, pnorm_lo, 1] x [-2c, 1, 1, cnorm]
    (pnorm split so the large norm term survives fp32r product rounding)
  - ACT sqrt batched over multi-tile PSUM groups; ACT exp batched as one
    giant instruction per chunk (sqrt/exp live in different ACT table
    sets; chunking keeps table reloads to 4 per launch)
  - softmax row-sums via batched DVE 3D tensor_reduce + reciprocal;
    normalization folded into the 19-wide pixel operand of the update
    matmul (never scales the [128,256] e-tile in iterations 0/1)
  - center update = (w*p_aug)^T e accumulated in PSUM; col 18 gives the
    softmax column sums for free
  - 3 SPMD launches (iter0, iter1: "update" NEFF; iter2: "final" NEFF
    that writes Q); the tiny [4,19,256] partials are reduced on host
    between launches.
"""
import os
import sys

sys.path.insert(0, "/opt/trn_rl_repo")

import numpy as np
from contextlib import ExitStack

import concourse.bass as bass
import concourse.bacc as bacc
import concourse.tile as tile
from concourse import mybir
from concourse.bass_utils import run_bass_kernel_spmd

AF = mybir.ActivationFunctionType
F32 = mybir.dt.float32
F32R = mybir.dt.float32r
F16 = mybir.dt.float16

B, C, H, W = 4, 16, 256, 256
N = H * W            # 65536 pixels per batch
M = 256              # superpixels
D = C + 2            # 18
NCORES = 8
NSH = N // NCORES    # 8192 pixels per batch per core
TPB = NSH // 128     # 64 tiles of 128 pixels per batch
TT = B * TPB         # 256 tiles per core per iteration
AUG = 21             # [p(18), pnorm_hi, pnorm_lo, 1]
UD = 19              # update dims [p(18), 1]
CHUNK = 128          # tiles per ACT-table phase chunk (2 chunks of 128)
GROUP = 6            # d2 tiles per PSUM group (3 banks, double buffered)

_kernel_cache = {}


def _build_nc(mode):
    """mode: 'update' (fp32r center partials), 'update2' (fp32-exact center
    partials, used for the iteration whose centers are returned), or
    'final' (emit Q)."""
    nc = bacc.Bacc("TRN2", target_bir_lowering=False, debug=False,
                   num_devices=NCORES)
    pT_d = nc.dram_tensor("pT", [128, NSH], F32R, kind="ExternalInput")
    cT_d = nc.dram_tensor("cT", [128, M], F32R, kind="ExternalInput")
    upd_mode = mode.startswith("update")
    if upd_mode:
        pu_d = nc.dram_tensor("pu", [128, TT * UD], F32, kind="ExternalInput")
        updp = 128 if mode == "update2" else UD
        upd_d = nc.dram_tensor("upd", [B, updp, M], F32, kind="ExternalOutput")
    else:
        q_d = nc.dram_tensor("q", [B, NSH, M], F32, kind="ExternalOutput")

    chunk_sizes = [TPB] * B if upd_mode else [96, 96, 64]

    with tile.TileContext(nc) as tc:
        with ExitStack() as ctx:
            const = ctx.enter_context(tc.tile_pool(name="const", bufs=1))
            dbufp = ctx.enter_context(tc.tile_pool(name="dist", bufs=1))
            rowp = ctx.enter_context(tc.tile_pool(name="rows", bufs=1))
            scr = ctx.enter_context(
                tc.tile_pool(name="scr", bufs=4 if upd_mode else 8))
            d2pool = ctx.enter_context(
                tc.tile_pool(name="d2ps", bufs=2, space="PSUM"))
            if upd_mode:
                updpool = ctx.enter_context(
                    tc.tile_pool(name="updps", bufs=2, space="PSUM"))

            cT = const.tile([128, M], F32R, tag="cT")
            nc.sync.dma_start(cT[:], cT_d.ap()[:])
            pT = const.tile([128, NSH], F32R, tag="pT")
            for q in range(16):  # split so first tiles' matmuls start early
                w0 = q * (NSH // 16)
                nc.sync.dma_start(pT[:, w0:w0 + NSH // 16],
                                  pT_d.ap()[:, w0:w0 + NSH // 16])
            if upd_mode:
                pu = const.tile([128, TT * UD], F32, tag="pu")
                step = TT * UD // 8
                for q in range(8):
                    nc.sync.dma_start(pu[:, q * step:(q + 1) * step],
                                      pu_d.ap()[:, q * step:(q + 1) * step])

            maxchunk = max(chunk_sizes)
            dist = dbufp.tile([128, maxchunk * M], F32, tag="dist")
            EDT = F32R if mode == "update" else F32
            if upd_mode:
                ebuf = dbufp.tile([128, maxchunk * M], EDT, tag="ebuf")
            rowsums = rowp.tile([128, maxchunk], F32, tag="rowsums")
            recips = rowp.tile([128, maxchunk], F32, tag="recips")

            prev_act = None

            def chain_act(inst):
                nonlocal prev_act
                if prev_act is not None:
                    tile.add_dep_helper(inst.ins, prev_act.ins, sync=False,
                                        reason="ACT table-set phase order")
                prev_act = inst

            base = 0
            for ci, chunk in enumerate(chunk_sizes):
                # --- phase A: d2 matmuls + batched sqrt (sqrt table set) ---
                grp = GROUP if upd_mode else 8  # final: updpool banks free
                for g0 in range(0, chunk, grp):
                    gsize = min(grp, chunk - g0)
                    ps = d2pool.tile([128, grp * M], F32, tag="d2")
                    with tc.high_priority():
                        # keep d2 fills ahead of update matmuls in PE order
                        for j in range(gsize):
                            i = base + g0 + j
                            b, t = divmod(i, TPB)
                            nc.tensor.matmul(
                                ps[:, j * M:(j + 1) * M],
                                pT[32 * b:32 * b + AUG,
                                   t * 128:(t + 1) * 128],
                                cT[32 * b:32 * b + AUG, :],
                                start=True, stop=True,
                                tile_position=(32 * b, 0))
                    off = g0 * M
                    chain_act(nc.scalar.activation(
                        dist[:, off:off + gsize * M],
                        ps[:, 0:gsize * M], AF.Sqrt))
                # --- phase B+C: exp in EB-tile pieces (exp table set); each
                #     piece's rowsums/recips/consumers emitted right after so
                #     DVE/PE/DMA pipeline behind the ACT stream ---
                EB = 8
                ebase = ebuf if upd_mode else dist
                if upd_mode:
                    b = ci  # chunk == TPB: one batch per chunk
                    # update2: 4 col-tiled fp32 accumulators at partition
                    # offsets 32j (summed on host); update: single fp32r
                    upd_ps = updpool.tile(
                        [128 if mode == "update2" else UD, M], F32,
                        tag="updps")
                for r0 in range(0, chunk, EB):
                    rn = min(EB, chunk - r0)
                    chain_act(nc.scalar.activation(
                        ebase[:, r0 * M:(r0 + rn) * M],
                        dist[:, r0 * M:(r0 + rn) * M], AF.Exp, scale=-1.0))
                    esrc = (ebase[:, r0 * M:(r0 + rn) * M].bitcast(F32)
                            if mode == "update"
                            else ebase[:, r0 * M:(r0 + rn) * M])
                    view = esrc.rearrange("p (s n) -> p s n", n=M)
                    nc.vector.tensor_reduce(
                        rowsums[:, r0:r0 + rn], view,
                        axis=mybir.AxisListType.X, op=mybir.AluOpType.add)
                    nc.vector.reciprocal(recips[:, r0:r0 + rn],
                                         rowsums[:, r0:r0 + rn])
                    if upd_mode:
                        for j in range(r0, r0 + rn):
                            t = j
                            pw = scr.tile([128, UD], EDT, tag="pw")
                            nc.vector.tensor_scalar_mul(
                                pw[:],
                                pu[:, (base + j) * UD:(base + j + 1) * UD],
                                recips[:, j:j + 1])
                            if mode == "update2":
                                cg = 32 * (t % 4)
                                nc.tensor.matmul(
                                    upd_ps[cg:cg + UD, :],
                                    pw[:],
                                    ebuf[:, j * M:(j + 1) * M],
                                    start=(t < 4), stop=(t >= TPB - 4),
                                    tile_position=(0, cg))
                            else:
                                nc.tensor.matmul(
                                    upd_ps[:],
                                    pw[:],
                                    ebuf[:, j * M:(j + 1) * M],
                                    start=(t == 0), stop=(t == TPB - 1))
                    else:
                        QB = 4
                        for j0 in range(r0, r0 + rn, QB):
                            stg = scr.tile([128, QB * M], F32, tag="qstg")
                            for j in range(j0, j0 + QB):
                                nc.vector.tensor_scalar_mul(
                                    stg[:, (j - j0) * M:(j - j0 + 1) * M],
                                    dist[:, j * M:(j + 1) * M],
                                    recips[:, j:j + 1])
                            i = base + j0
                            b, t = divmod(i, TPB)
                            dram = q_d.ap()[b, t * 128:(t + QB) * 128, :] \
                                .rearrange("(s p) n -> p s n", p=128)
                            nc.sync.dma_start(
                                dram,
                                stg[:].rearrange("p (s n) -> p s n", n=M))
                if upd_mode:
                    pdim = 128 if mode == "update2" else UD
                    upd_sb = scr.tile([pdim, M], F32, tag="updsb")
                    nc.vector.tensor_copy(upd_sb[:], upd_ps[:])
                    nc.sync.dma_start(upd_d.ap()[b], upd_sb[:])
                base += chunk
    nc.finalize()
    return nc


def _get_nc(mode):
    if mode not in _kernel_cache:
        _kernel_cache[mode] = _build_nc(mode)
    return _kernel_cache[mode]


def _trunc11(v):
    """Truncate fp32 mantissa to 11 explicit bits (exactly fp32r-representable)."""
    u = v.view(np.uint32) & np.uint32(0xFFFFF000)
    return u.view(np.float32)


def _make_ctilde(c):
    """c: [B, M, D] fp32 -> packed cT input [128, M] (batch b at rows 32b..)."""
    cn = (c.astype(np.float64) ** 2).sum(-1).astype(np.float32)  # [B, M]
    a = np.zeros((128, M), np.float32)
    for b in range(B):
        a[32 * b + 0:32 * b + D] = (-2.0 * c[b].T)
        a[32 * b + D] = 1.0
        a[32 * b + D + 1] = 1.0
        a[32 * b + D + 2] = cn[b]
    return np.ascontiguousarray(a)


def _run(nc, in_maps, trace=False):
    res = run_bass_kernel_spmd(nc, in_maps, core_ids=list(range(NCORES)),
                               trace=trace)
    return res


def kernel(features):
    features = np.asarray(features, dtype=np.float32)
    assert features.shape == (B, C, H, W)

    # ---- host prep: pixel vectors (exactly as reference) ----
    y = (np.arange(H, dtype=np.float32) / np.float32(H))
    x = (np.arange(W, dtype=np.float32) / np.float32(W))
    mesh_y, mesh_x = np.meshgrid(y, x, indexing="ij")
    p = np.empty((B, N, D), np.float32)
    p[:, :, :C] = features.reshape(B, C, N).transpose(0, 2, 1)
    p[:, :, C] = mesh_x.reshape(N)[None, :]
    p[:, :, C + 1] = mesh_y.reshape(N)[None, :]

    pnorm = (p.astype(np.float64) ** 2).sum(-1).astype(np.float32)  # [B, N]
    pn_hi = _trunc11(pnorm)
    pn_lo = pnorm - pn_hi

    # initial centers: 16x16 block means of [features; coords]
    s = 16
    blk = p.reshape(B, s, H // s, s, W // s, D)
    c = blk.mean(axis=(2, 4), dtype=np.float32).reshape(B, M, D)

    # ---- per-core static inputs ----
    pT_core = []   # [core] -> [128, NSH] packed (batch b at rows 32b..)
    pu_core = []   # [core] -> [128, TT*UD]
    for k in range(NCORES):
        sl = slice(k * NSH, (k + 1) * NSH)
        a = np.zeros((128, NSH), np.float32)
        for b in range(B):
            a[32 * b + 0:32 * b + D] = p[b, sl].T
            a[32 * b + D] = pn_hi[b, sl]
            a[32 * b + D + 1] = pn_lo[b, sl]
            a[32 * b + D + 2] = 1.0
        pT_core.append(np.ascontiguousarray(a))
        pu = np.empty((B, TPB, 128, UD), np.float32)
        pu[:, :, :, :D] = p[:, sl].reshape(B, TPB, 128, D)
        pu[:, :, :, D] = 1.0
        pu = pu.transpose(2, 0, 1, 3).reshape(128, TT * UD)
        pu_core.append(np.ascontiguousarray(pu))

    # ---- iterations 0 and 1: center updates (iter 1 exact fp32: its
    #      centers are the returned centers_feat) ----
    for it in range(2):
        nc_upd = _get_nc("update" if it == 0 else "update2")
        ct = _make_ctilde(c)
        in_maps = [{"pT": pT_core[k], "cT": ct, "pu": pu_core[k]}
                   for k in range(NCORES)]
        res = _run(nc_upd, in_maps)
        upd = np.zeros((B, UD, M), np.float64)
        for k in range(NCORES):
            u = res.results[k]["upd"].astype(np.float64)
            if u.shape[1] == 128:  # col-tiled: sum the 4 partition slices
                u = sum(u[:, 32 * j:32 * j + UD, :] for j in range(4))
            upd += u
        colsum = upd[:, D, :]                       # [B, M]
        c = (upd[:, :D, :] / (colsum[:, None, :] + 1e-6)) \
            .transpose(0, 2, 1).astype(np.float32)  # [B, M, D]

    # ---- final iteration: Q ----
    nc_fin = _get_nc("final")
    ct = _make_ctilde(c)
    in_maps = [{"pT": pT_core[k], "cT": ct} for k in range(NCORES)]
    res = _run(nc_fin, in_maps)
    Q = np.empty((B, N, M), np.float32)
    for k in range(NCORES):
        Q[:, k * NSH:(k + 1) * NSH, :] = res.results[k]["q"]

    centers_feat = np.ascontiguousarray(c[:, :, :C].transpose(0, 2, 1))
    return Q, centers_feat


if __name__ == "__main__":
    feats = np.random.default_rng(0).standard_normal(
        (B, C, H, W)).astype(np.float32)
    Q, cf = kernel(feats)
    print("Q", Q.shape, Q.dtype, "centers_feat", cf.shape)


# revision 27
# speedup vs baseline: 1.0019x; 1.0019x over previous
"""Differentiable SSN (superpixel) kernel for Trainium2, 8 NeuronCores.

Computes 3 iterations of {cdist -> softmax -> soft center update} for
B=4, N=65536 pixels, M=256 centers, D=18 (16 features + 2 coords).

Strategy:
  - pixels sharded 8 ways across cores (each core: 8192 pixels x 4 batches)
  - distance^2 via one fp32r matmul per 128-pixel tile using a 21-dim
    augmentation [p, pnorm_hi, pnorm_lo, 1] x [-2c, 1, 1, cnorm]
    (pnorm split so the large norm term survives fp32r product rounding)
  - ACT sqrt batched over multi-tile PSUM groups; ACT exp batched as one
    giant instruction per chunk (sqrt/exp live in different ACT table
    sets; chunking keeps table reloads to 4 per launch)
  - softmax row-sums via batched DVE 3D tensor_reduce + reciprocal;
    normalization folded into the 19-wide pixel operand of the update
    matmul (never scales the [128,256] e-tile in iterations 0/1)
  - center update = (w*p_aug)^T e accumulated in PSUM; col 18 gives the
    softmax column sums for free
  - 3 SPMD launches (iter0, iter1: "update" NEFF; iter2: "final" NEFF
    that writes Q); the tiny [4,19,256] partials are reduced on host
    between launches.
"""
import os
import sys

sys.path.insert(0, "/opt/trn_rl_repo")

import numpy as np
from contextlib import ExitStack

import concourse.bass as bass
import concourse.bacc as bacc
import concourse.tile as tile
from concourse import mybir
from concourse.bass_utils import run_bass_kernel_spmd

AF = mybir.ActivationFunctionType
F32 = mybir.dt.float32
F32R = mybir.dt.float32r
F16 = mybir.dt.float16

B, C, H, W = 4, 16, 256, 256
N = H * W            # 65536 pixels per batch
M = 256              # superpixels
D = C + 2            # 18
NCORES = 8
NSH = N // NCORES    # 8192 pixels per batch per core
TPB = NSH // 128     # 64 tiles of 128 pixels per batch
TT = B * TPB         # 256 tiles per core per iteration
AUG = 21             # [p(18), pnorm_hi, pnorm_lo, 1]
UD = 19              # update dims [p(18), 1]
CHUNK = 128          # tiles per ACT-table phase chunk (2 chunks of 128)
GROUP = 6            # d2 tiles per PSUM group (3 banks, double buffered)

_kernel_cache = {}


def _build_nc(mode):
    """mode: 'update' (fp32r center partials), 'update2' (fp32-exact center
    partials, used for the iteration whose centers are returned), or
    'final' (emit Q)."""
    nc = bacc.Bacc("TRN2", target_bir_lowering=False, debug=False,
                   num_devices=NCORES)
    pT_d = nc.dram_tensor("pT", [128, NSH], F32R, kind="ExternalInput")
    cT_d = nc.dram_tensor("cT", [128, M], F32R, kind="ExternalInput")
    upd_mode = mode.startswith("update")
    if upd_mode:
        pu_d = nc.dram_tensor("pu", [128, TT * UD], F32, kind="ExternalInput")
        updp = 128 if mode == "update2" else UD
        upd_d = nc.dram_tensor("upd", [B, updp, M], F32, kind="ExternalOutput")
    else:
        q_d = nc.dram_tensor("q", [B, NSH, M], F32, kind="ExternalOutput")

    chunk_sizes = [TPB] * B if upd_mode else [96, 96, 64]

    with tile.TileContext(nc) as tc:
        with ExitStack() as ctx:
            const = ctx.enter_context(tc.tile_pool(name="const", bufs=1))
            dbufp = ctx.enter_context(tc.tile_pool(name="dist", bufs=1))
            rowp = ctx.enter_context(tc.tile_pool(name="rows", bufs=1))
            scr = ctx.enter_context(
                tc.tile_pool(name="scr", bufs=4 if upd_mode else 8))
            d2pool = ctx.enter_context(
                tc.tile_pool(name="d2ps", bufs=2, space="PSUM"))
            if upd_mode:
                updpool = ctx.enter_context(
                    tc.tile_pool(name="updps", bufs=2, space="PSUM"))

            cT = const.tile([128, M], F32R, tag="cT")
            nc.sync.dma_start(cT[:], cT_d.ap()[:])
            pT = const.tile([128, NSH], F32R, tag="pT")
            # tiny first piece so the first d2 matmuls unblock asap
            pieces = [(0, 128), (128, 384), (512, 512)] + \
                     [(w, 1024) for w in range(1024, NSH, 1024)]
            for w0, wn in pieces:
                nc.sync.dma_start(pT[:, w0:w0 + wn],
                                  pT_d.ap()[:, w0:w0 + wn])
            if upd_mode:
                pu = const.tile([128, TT * UD], F32, tag="pu")
                step = TT * UD // 8
                for q in range(8):
                    nc.sync.dma_start(pu[:, q * step:(q + 1) * step],
                                      pu_d.ap()[:, q * step:(q + 1) * step])

            maxchunk = max(chunk_sizes)
            dist = dbufp.tile([128, maxchunk * M], F32, tag="dist")
            EDT = F32R if mode == "update" else F32
            if upd_mode:
                ebuf = dbufp.tile([128, maxchunk * M], EDT, tag="ebuf")
            rowsums = rowp.tile([128, maxchunk], F32, tag="rowsums")
            recips = rowp.tile([128, maxchunk], F32, tag="recips")

            prev_act = None

            def chain_act(inst):
                nonlocal prev_act
                if prev_act is not None:
                    tile.add_dep_helper(inst.ins, prev_act.ins, sync=False,
                                        reason="ACT table-set phase order")
                prev_act = inst

            base = 0
            for ci, chunk in enumerate(chunk_sizes):
                # --- phase A: d2 matmuls + batched sqrt (sqrt table set) ---
                grp = GROUP if upd_mode else 8  # final: updpool banks free
                for g0 in range(0, chunk, grp):
                    gsize = min(grp, chunk - g0)
                    ps = d2pool.tile([128, grp * M], F32, tag="d2")
                    with tc.high_priority():
                        # keep d2 fills ahead of update matmuls in PE order
                        for j in range(gsize):
                            i = base + g0 + j
                            b, t = divmod(i, TPB)
                            nc.tensor.matmul(
                                ps[:, j * M:(j + 1) * M],
                                pT[32 * b:32 * b + AUG,
                                   t * 128:(t + 1) * 128],
                                cT[32 * b:32 * b + AUG, :],
                                start=True, stop=True,
                                tile_position=(32 * b, 0))
                    off = g0 * M
                    chain_act(nc.scalar.activation(
                        dist[:, off:off + gsize * M],
                        ps[:, 0:gsize * M], AF.Sqrt))
                # --- phase B+C: exp in EB-tile pieces (exp table set); each
                #     piece's rowsums/recips/consumers emitted right after so
                #     DVE/PE/DMA pipeline behind the ACT stream ---
                EB = 8
                ebase = ebuf if upd_mode else dist
                if upd_mode:
                    b = ci  # chunk == TPB: one batch per chunk
                    # update2: 4 col-tiled fp32 accumulators at partition
                    # offsets 32j (summed on host); update: single fp32r
                    upd_ps = updpool.tile(
                        [128 if mode == "update2" else UD, M], F32,
                        tag="updps")
                for r0 in range(0, chunk, EB):
                    rn = min(EB, chunk - r0)
                    chain_act(nc.scalar.activation(
                        ebase[:, r0 * M:(r0 + rn) * M],
                        dist[:, r0 * M:(r0 + rn) * M], AF.Exp, scale=-1.0))
                    esrc = (ebase[:, r0 * M:(r0 + rn) * M].bitcast(F32)
                            if mode == "update"
                            else ebase[:, r0 * M:(r0 + rn) * M])
                    view = esrc.rearrange("p (s n) -> p s n", n=M)
                    nc.vector.tensor_reduce(
                        rowsums[:, r0:r0 + rn], view,
                        axis=mybir.AxisListType.X, op=mybir.AluOpType.add)
                    nc.vector.reciprocal(recips[:, r0:r0 + rn],
                                         rowsums[:, r0:r0 + rn])
                    if upd_mode:
                        for j in range(r0, r0 + rn):
                            t = j
                            pw = scr.tile([128, UD], EDT, tag="pw")
                            nc.vector.tensor_scalar_mul(
                                pw[:],
                                pu[:, (base + j) * UD:(base + j + 1) * UD],
                                recips[:, j:j + 1])
                            if mode == "update2":
                                cg = 32 * (t % 4)
                                nc.tensor.matmul(
                                    upd_ps[cg:cg + UD, :],
                                    pw[:],
                                    ebuf[:, j * M:(j + 1) * M],
                                    start=(t < 4), stop=(t >= TPB - 4),
                                    tile_position=(0, cg))
                            else:
                                nc.tensor.matmul(
                                    upd_ps[:],
                                    pw[:],
                                    ebuf[:, j * M:(j + 1) * M],
                                    start=(t == 0), stop=(t == TPB - 1))
                    else:
                        QB = 4
                        for j0 in range(r0, r0 + rn, QB):
                            stg = scr.tile([128, QB * M], F32, tag="qstg")
                            for j in range(j0, j0 + QB):
                                nc.vector.tensor_scalar_mul(
                                    stg[:, (j - j0) * M:(j - j0 + 1) * M],
                                    dist[:, j * M:(j + 1) * M],
                                    recips[:, j:j + 1])
                            i = base + j0
                            b, t = divmod(i, TPB)
                            dram = q_d.ap()[b, t * 128:(t + QB) * 128, :] \
                                .rearrange("(s p) n -> p s n", p=128)
                            nc.sync.dma_start(
                                dram,
                                stg[:].rearrange("p (s n) -> p s n", n=M))
                if upd_mode:
                    pdim = 128 if mode == "update2" else UD
                    upd_sb = scr.tile([pdim, M], F32, tag="updsb")
                    nc.vector.tensor_copy(upd_sb[:], upd_ps[:])
                    nc.sync.dma_start(upd_d.ap()[b], upd_sb[:])
                base += chunk
    nc.finalize()
    return nc


def _get_nc(mode):
    if mode not in _kernel_cache:
        _kernel_cache[mode] = _build_nc(mode)
    return _kernel_cache[mode]


def _trunc11(v):
    """Truncate fp32 mantissa to 11 explicit bits (exactly fp32r-representable)."""
    u = v.view(np.uint32) & np.uint32(0xFFFFF000)
    return u.view(np.float32)


def _make_ctilde(c):
    """c: [B, M, D] fp32 -> packed cT input [128, M] (batch b at rows 32b..)."""
    cn = (c.astype(np.float64) ** 2).sum(-1).astype(np.float32)  # [B, M]
    a = np.zeros((128, M), np.float32)
    for b in range(B):
        a[32 * b + 0:32 * b + D] = (-2.0 * c[b].T)
        a[32 * b + D] = 1.0
        a[32 * b + D + 1] = 1.0
        a[32 * b + D + 2] = cn[b]
    return np.ascontiguousarray(a)


def _run(nc, in_maps, trace=False):
    res = run_bass_kernel_spmd(nc, in_maps, core_ids=list(range(NCORES)),
                               trace=trace)
    return res


def kernel(features):
    features = np.asarray(features, dtype=np.float32)
    assert features.shape == (B, C, H, W)

    # ---- host prep: pixel vectors (exactly as reference) ----
    y = (np.arange(H, dtype=np.float32) / np.float32(H))
    x = (np.arange(W, dtype=np.float32) / np.float32(W))
    mesh_y, mesh_x = np.meshgrid(y, x, indexing="ij")
    p = np.empty((B, N, D), np.float32)
    p[:, :, :C] = features.reshape(B, C, N).transpose(0, 2, 1)
    p[:, :, C] = mesh_x.reshape(N)[None, :]
    p[:, :, C + 1] = mesh_y.reshape(N)[None, :]

    pnorm = (p.astype(np.float64) ** 2).sum(-1).astype(np.float32)  # [B, N]
    pn_hi = _trunc11(pnorm)
    pn_lo = pnorm - pn_hi

    # initial centers: 16x16 block means of [features; coords]
    s = 16
    blk = p.reshape(B, s, H // s, s, W // s, D)
    c = blk.mean(axis=(2, 4), dtype=np.float32).reshape(B, M, D)

    # ---- per-core static inputs ----
    pT_core = []   # [core] -> [128, NSH] packed (batch b at rows 32b..)
    pu_core = []   # [core] -> [128, TT*UD]
    for k in range(NCORES):
        sl = slice(k * NSH, (k + 1) * NSH)
        a = np.zeros((128, NSH), np.float32)
        for b in range(B):
            a[32 * b + 0:32 * b + D] = p[b, sl].T
            a[32 * b + D] = pn_hi[b, sl]
            a[32 * b + D + 1] = pn_lo[b, sl]
            a[32 * b + D + 2] = 1.0
        pT_core.append(np.ascontiguousarray(a))
        pu = np.empty((B, TPB, 128, UD), np.float32)
        pu[:, :, :, :D] = p[:, sl].reshape(B, TPB, 128, D)
        pu[:, :, :, D] = 1.0
        pu = pu.transpose(2, 0, 1, 3).reshape(128, TT * UD)
        pu_core.append(np.ascontiguousarray(pu))

    # ---- iterations 0 and 1: center updates (iter 1 exact fp32: its
    #      centers are the returned centers_feat) ----
    for it in range(2):
        nc_upd = _get_nc("update" if it == 0 else "update2")
        ct = _make_ctilde(c)
        in_maps = [{"pT": pT_core[k], "cT": ct, "pu": pu_core[k]}
                   for k in range(NCORES)]
        res = _run(nc_upd, in_maps)
        upd = np.zeros((B, UD, M), np.float64)
        for k in range(NCORES):
            u = res.results[k]["upd"].astype(np.float64)
            if u.shape[1] == 128:  # col-tiled: sum the 4 partition slices
                u = sum(u[:, 32 * j:32 * j + UD, :] for j in range(4))
            upd += u
        colsum = upd[:, D, :]                       # [B, M]
        c = (upd[:, :D, :] / (colsum[:, None, :] + 1e-6)) \
            .transpose(0, 2, 1).astype(np.float32)  # [B, M, D]

    # ---- final iteration: Q ----
    nc_fin = _get_nc("final")
    ct = _make_ctilde(c)
    in_maps = [{"pT": pT_core[k], "cT": ct} for k in range(NCORES)]
    res = _run(nc_fin, in_maps)
    Q = np.empty((B, N, M), np.float32)
    for k in range(NCORES):
        Q[:, k * NSH:(k + 1) * NSH, :] = res.results[k]["q"]

    centers_feat = np.ascontiguousarray(c[:, :, :C].transpose(0, 2, 1))
    return Q, centers_feat


if __name__ == "__main__":
    feats = np.random.default_rng(0).standard_normal(
        (B, C, H, W)).astype(np.float32)
    Q, cf = kernel(feats)
    print("Q", Q.shape, Q.dtype, "centers_feat", cf.shape)


# revision 29
# speedup vs baseline: 1.0443x; 1.0423x over previous
"""Differentiable SSN (superpixel) kernel for Trainium2, 8 NeuronCores.

Computes 3 iterations of {cdist -> softmax -> soft center update} for
B=4, N=65536 pixels, M=256 centers, D=18 (16 features + 2 coords).

Strategy:
  - pixels sharded 8 ways across cores (each core: 8192 pixels x 4 batches)
  - distance^2 via one fp32r matmul per 128-pixel tile using a 21-dim
    augmentation [p, pnorm_hi, pnorm_lo, 1] x [-2c, 1, 1, cnorm]
    (pnorm split so the large norm term survives fp32r product rounding)
  - ACT sqrt batched over 6-8-tile PSUM groups; ACT exp in 8-tile pieces
    (sqrt/exp live in different ACT table sets, ~2.7us/switch, so work is
    phased in 64-96-tile chunks with an explicit ACT-order chain); each
    piece's row-sums (DVE 3D tensor_reduce) + reciprocal + consumers are
    emitted per piece so DVE/PE/DMA pipeline behind the ACT stream
  - softmax normalization folded into the 19-wide pixel operand of the
    update matmul (never scales the [128,256] e-tile in iterations 0/1);
    center update = (w*p_aug)^T e accumulated in PSUM, col 18 = colsums
  - iter0 update in fp32r (1 cyc/row); iter1 (whose centers are returned)
    in exact fp32 via 4-way column-tiled matmuls (tile_position=(0,32j),
    four PSUM accumulators at partition offsets 32j, summed on host)
  - final launch scales Q into staging tiles and ships 4 tiles per DMA;
    3 SPMD launches, tiny center partials reduced on host between them.
"""
import os
import sys

sys.path.insert(0, "/opt/trn_rl_repo")

import numpy as np
from contextlib import ExitStack

import concourse.bass as bass
import concourse.bacc as bacc
import concourse.tile as tile
from concourse import mybir
from concourse.bass_utils import run_bass_kernel_spmd

AF = mybir.ActivationFunctionType
F32 = mybir.dt.float32
F32R = mybir.dt.float32r
F16 = mybir.dt.float16

B, C, H, W = 4, 16, 256, 256
N = H * W            # 65536 pixels per batch
M = 256              # superpixels
D = C + 2            # 18
NCORES = 8
NSH = N // NCORES    # 8192 pixels per batch per core
TPB = NSH // 128     # 64 tiles of 128 pixels per batch
TT = B * TPB         # 256 tiles per core per iteration
AUG = 21             # [p(18), pnorm_hi, pnorm_lo, 1]
UD = 19              # update dims [p(18), 1]
CHUNK = 128          # tiles per ACT-table phase chunk (2 chunks of 128)
GROUP = 6            # d2 tiles per PSUM group (3 banks, double buffered)

_kernel_cache = {}


def _build_nc(mode):
    """mode: 'update' (fp32r center partials), 'update2' (fp32-exact center
    partials, used for the iteration whose centers are returned), or
    'final' (emit Q)."""
    nc = bacc.Bacc("TRN2", target_bir_lowering=False, debug=False,
                   num_devices=NCORES)
    pT_d = nc.dram_tensor("pT", [128, NSH], F32R, kind="ExternalInput")
    cT_d = nc.dram_tensor("cT", [128, M], F32R, kind="ExternalInput")
    upd_mode = mode.startswith("update")
    if upd_mode:
        pu_d = nc.dram_tensor("pu", [128, TT * UD], F32, kind="ExternalInput")
        updp = 128 if mode == "update2" else UD
        upd_d = nc.dram_tensor("upd", [B, updp, M], F32, kind="ExternalOutput")
    else:
        q_d = nc.dram_tensor("q", [B, NSH, M], F32, kind="ExternalOutput")

    chunk_sizes = [TPB] * B if upd_mode else [96, 96, 64]

    with tile.TileContext(nc) as tc:
        with ExitStack() as ctx:
            const = ctx.enter_context(tc.tile_pool(name="const", bufs=1))
            dbufp = ctx.enter_context(tc.tile_pool(name="dist", bufs=1))
            rowp = ctx.enter_context(tc.tile_pool(name="rows", bufs=1))
            scr = ctx.enter_context(
                tc.tile_pool(name="scr", bufs=8))
            d2pool = ctx.enter_context(
                tc.tile_pool(name="d2ps", bufs=2, space="PSUM"))
            if upd_mode:
                updpool = ctx.enter_context(
                    tc.tile_pool(name="updps", bufs=2, space="PSUM"))

            cT = const.tile([128, M], F32R, tag="cT")
            nc.sync.dma_start(cT[:], cT_d.ap()[:])
            pT = const.tile([128, NSH], F32R, tag="pT")
            # tiny first piece so the first d2 matmuls unblock asap
            pieces = [(0, 128), (128, 384), (512, 512)] + \
                     [(w, 1024) for w in range(1024, NSH, 1024)]
            for w0, wn in pieces:
                nc.sync.dma_start(pT[:, w0:w0 + wn],
                                  pT_d.ap()[:, w0:w0 + wn])
            if upd_mode:
                pu = const.tile([128, TT * UD], F32, tag="pu")
                step = TT * UD // 8
                for q in range(8):
                    nc.sync.dma_start(pu[:, q * step:(q + 1) * step],
                                      pu_d.ap()[:, q * step:(q + 1) * step])

            maxchunk = max(chunk_sizes)
            dist = dbufp.tile([128, maxchunk * M], F32, tag="dist")
            EDT = F32R if mode == "update" else F32
            if upd_mode:
                ebuf = dbufp.tile([128, maxchunk * M], EDT, tag="ebuf")
            rowsums = rowp.tile([128, maxchunk], F32, tag="rowsums")
            recips = rowp.tile([128, maxchunk], F32, tag="recips")

            prev_act = None

            def chain_act(inst):
                nonlocal prev_act
                if prev_act is not None:
                    tile.add_dep_helper(inst.ins, prev_act.ins, sync=False,
                                        reason="ACT table-set phase order")
                prev_act = inst

            base = 0
            for ci, chunk in enumerate(chunk_sizes):
                # --- phase A: d2 matmuls + batched sqrt (sqrt table set) ---
                grp = GROUP if upd_mode else 8  # final: updpool banks free
                for g0 in range(0, chunk, grp):
                    gsize = min(grp, chunk - g0)
                    ps = d2pool.tile([128, grp * M], F32, tag="d2")
                    with tc.high_priority():
                        # keep d2 fills ahead of update matmuls in PE order
                        for j in range(gsize):
                            i = base + g0 + j
                            b, t = divmod(i, TPB)
                            nc.tensor.matmul(
                                ps[:, j * M:(j + 1) * M],
                                pT[32 * b:32 * b + AUG,
                                   t * 128:(t + 1) * 128],
                                cT[32 * b:32 * b + AUG, :],
                                start=True, stop=True,
                                tile_position=(32 * b, 0))
                    off = g0 * M
                    chain_act(nc.scalar.activation(
                        dist[:, off:off + gsize * M],
                        ps[:, 0:gsize * M], AF.Sqrt))
                # --- phase B+C: exp in EB-tile pieces (exp table set); each
                #     piece's rowsums/recips/consumers emitted right after so
                #     DVE/PE/DMA pipeline behind the ACT stream ---
                EB = 8
                ebase = ebuf if upd_mode else dist
                if upd_mode:
                    b = ci  # chunk == TPB: one batch per chunk
                    # update2: 4 col-tiled fp32 accumulators at partition
                    # offsets 32j (summed on host); update: single fp32r
                    upd_ps = updpool.tile(
                        [128 if mode == "update2" else UD, M], F32,
                        tag="updps")
                for r0 in range(0, chunk, EB):
                    rn = min(EB, chunk - r0)
                    chain_act(nc.scalar.activation(
                        ebase[:, r0 * M:(r0 + rn) * M],
                        dist[:, r0 * M:(r0 + rn) * M], AF.Exp, scale=-1.0))
                    esrc = (ebase[:, r0 * M:(r0 + rn) * M].bitcast(F32)
                            if mode == "update"
                            else ebase[:, r0 * M:(r0 + rn) * M])
                    view = esrc.rearrange("p (s n) -> p s n", n=M)
                    nc.vector.tensor_reduce(
                        rowsums[:, r0:r0 + rn], view,
                        axis=mybir.AxisListType.X, op=mybir.AluOpType.add)
                    nc.vector.reciprocal(recips[:, r0:r0 + rn],
                                         rowsums[:, r0:r0 + rn])
                    if upd_mode:
                        for j in range(r0, r0 + rn):
                            t = j
                            pw = scr.tile([128, UD], EDT, tag="pw")
                            nc.vector.tensor_scalar_mul(
                                pw[:],
                                pu[:, (base + j) * UD:(base + j + 1) * UD],
                                recips[:, j:j + 1])
                            if mode == "update2":
                                cg = 32 * (t % 4)
                                nc.tensor.matmul(
                                    upd_ps[cg:cg + UD, :],
                                    pw[:],
                                    ebuf[:, j * M:(j + 1) * M],
                                    start=(t < 4), stop=(t >= TPB - 4),
                                    tile_position=(0, cg))
                            else:
                                nc.tensor.matmul(
                                    upd_ps[:],
                                    pw[:],
                                    ebuf[:, j * M:(j + 1) * M],
                                    start=(t == 0), stop=(t == TPB - 1))
                    else:
                        QB = 4
                        for j0 in range(r0, r0 + rn, QB):
                            stg = scr.tile([128, QB * M], F32, tag="qstg")
                            for j in range(j0, j0 + QB):
                                nc.vector.tensor_scalar_mul(
                                    stg[:, (j - j0) * M:(j - j0 + 1) * M],
                                    dist[:, j * M:(j + 1) * M],
                                    recips[:, j:j + 1])
                            i = base + j0
                            b, t = divmod(i, TPB)
                            dram = q_d.ap()[b, t * 128:(t + QB) * 128, :] \
                                .rearrange("(s p) n -> p s n", p=128)
                            nc.sync.dma_start(
                                dram,
                                stg[:].rearrange("p (s n) -> p s n", n=M))
                if upd_mode:
                    pdim = 128 if mode == "update2" else UD
                    upd_sb = scr.tile([pdim, M], F32, tag="updsb")
                    nc.vector.tensor_copy(upd_sb[:], upd_ps[:])
                    nc.sync.dma_start(upd_d.ap()[b], upd_sb[:])
                base += chunk
    nc.finalize()
    return nc


def _get_nc(mode):
    if mode not in _kernel_cache:
        _kernel_cache[mode] = _build_nc(mode)
    return _kernel_cache[mode]


def _trunc11(v):
    """Truncate fp32 mantissa to 11 explicit bits (exactly fp32r-representable)."""
    u = v.view(np.uint32) & np.uint32(0xFFFFF000)
    return u.view(np.float32)


def _make_ctilde(c):
    """c: [B, M, D] fp32 -> packed cT input [128, M] (batch b at rows 32b..)."""
    cn = (c.astype(np.float64) ** 2).sum(-1).astype(np.float32)  # [B, M]
    a = np.zeros((128, M), np.float32)
    for b in range(B):
        a[32 * b + 0:32 * b + D] = (-2.0 * c[b].T)
        a[32 * b + D] = 1.0
        a[32 * b + D + 1] = 1.0
        a[32 * b + D + 2] = cn[b]
    return np.ascontiguousarray(a)


def _run(nc, in_maps, trace=False):
    res = run_bass_kernel_spmd(nc, in_maps, core_ids=list(range(NCORES)),
                               trace=trace)
    return res


def kernel(features):
    features = np.asarray(features, dtype=np.float32)
    assert features.shape == (B, C, H, W)

    # ---- host prep: pixel vectors (exactly as reference) ----
    y = (np.arange(H, dtype=np.float32) / np.float32(H))
    x = (np.arange(W, dtype=np.float32) / np.float32(W))
    mesh_y, mesh_x = np.meshgrid(y, x, indexing="ij")
    p = np.empty((B, N, D), np.float32)
    p[:, :, :C] = features.reshape(B, C, N).transpose(0, 2, 1)
    p[:, :, C] = mesh_x.reshape(N)[None, :]
    p[:, :, C + 1] = mesh_y.reshape(N)[None, :]

    pnorm = (p.astype(np.float64) ** 2).sum(-1).astype(np.float32)  # [B, N]
    pn_hi = _trunc11(pnorm)
    pn_lo = pnorm - pn_hi

    # initial centers: 16x16 block means of [features; coords]
    s = 16
    blk = p.reshape(B, s, H // s, s, W // s, D)
    c = blk.mean(axis=(2, 4), dtype=np.float32).reshape(B, M, D)

    # ---- per-core static inputs ----
    pT_core = []   # [core] -> [128, NSH] packed (batch b at rows 32b..)
    pu_core = []   # [core] -> [128, TT*UD]
    for k in range(NCORES):
        sl = slice(k * NSH, (k + 1) * NSH)
        a = np.zeros((128, NSH), np.float32)
        for b in range(B):
            a[32 * b + 0:32 * b + D] = p[b, sl].T
            a[32 * b + D] = pn_hi[b, sl]
            a[32 * b + D + 1] = pn_lo[b, sl]
            a[32 * b + D + 2] = 1.0
        pT_core.append(np.ascontiguousarray(a))
        pu = np.empty((B, TPB, 128, UD), np.float32)
        pu[:, :, :, :D] = p[:, sl].reshape(B, TPB, 128, D)
        pu[:, :, :, D] = 1.0
        pu = pu.transpose(2, 0, 1, 3).reshape(128, TT * UD)
        pu_core.append(np.ascontiguousarray(pu))

    # ---- iterations 0 and 1: center updates (iter 1 exact fp32: its
    #      centers are the returned centers_feat) ----
    for it in range(2):
        nc_upd = _get_nc("update" if it == 0 else "update2")
        ct = _make_ctilde(c)
        in_maps = [{"pT": pT_core[k], "cT": ct, "pu": pu_core[k]}
                   for k in range(NCORES)]
        res = _run(nc_upd, in_maps)
        upd = np.zeros((B, UD, M), np.float64)
        for k in range(NCORES):
            u = res.results[k]["upd"].astype(np.float64)
            if u.shape[1] == 128:  # col-tiled: sum the 4 partition slices
                u = sum(u[:, 32 * j:32 * j + UD, :] for j in range(4))
            upd += u
        colsum = upd[:, D, :]                       # [B, M]
        c = (upd[:, :D, :] / (colsum[:, None, :] + 1e-6)) \
            .transpose(0, 2, 1).astype(np.float32)  # [B, M, D]

    # ---- final iteration: Q ----
    nc_fin = _get_nc("final")
    ct = _make_ctilde(c)
    in_maps = [{"pT": pT_core[k], "cT": ct} for k in range(NCORES)]
    res = _run(nc_fin, in_maps)
    Q = np.empty((B, N, M), np.float32)
    for k in range(NCORES):
        Q[:, k * NSH:(k + 1) * NSH, :] = res.results[k]["q"]

    centers_feat = np.ascontiguousarray(c[:, :, :C].transpose(0, 2, 1))
    return Q, centers_feat


if __name__ == "__main__":
    feats = np.random.default_rng(0).standard_normal(
        (B, C, H, W)).astype(np.float32)
    Q, cf = kernel(feats)
    print("Q", Q.shape, Q.dtype, "centers_feat", cf.shape)
